# revision 92
# baseline (speedup 1.0000x reference)
"""Bass/Trainium2 kernel for nn_NodesToEdges (gnn_message_passing).

out[b,i,j,:] = rms(edges[b,i,j,:])*g_e @ We + rms(nodes[b,i,:])*g_n @ Wr
             + rms(nodes[b,j,:])*g_n @ Wc + bias

Strategy: shard over i (rows) across 8 cores. The tiny node path
(row_proj / col_proj) is precomputed on host in numpy. The edge path
(134 MB in / 134 MB out, memory-bound) runs on device.

Per 512-row block pair (2 blocks packed on 128 partitions, 8 j-rows
per partition, j-layout cols = (r, e)):

  sq = x^2 (ACT) -> ss = group-reduce (DVE) -> rms = sqrt(ss/64+eps)
  (ACT) -> inv = 1/rms (DVE) -> xs = x*inv, cast f16 (Pool) -> PE f16
  transpose per 128-col tile -> xT sbuf (ACT copy) -> PE: psum primed
  with row_proj (rank-2 f32r matmul vs block indicator), then per-tile
  f16 matmul with lhsT=xT_tile, rhs=block-diag Wg accumulates the edge
  projection DIRECTLY in j-layout (no back-transpose, no second psum
  round-trip) -> final = psum + (colproj+bias) (DVE) -> DMA out.

Input/output DMAs move 2 pairs (512KB, 2KB contiguous lines) each and
all issue on SP. Every cross-engine producer->consumer hop is given a
full pipeline-iteration of slack (10-stage skew ladder) so semaphore
latency (~300-400ns/hop) never enters the steady-state recurrence; the
out-dma issue trails by 3 extra iterations so SP's wait on the first
finals never head-of-line-blocks input prefetch. Steady state is HBM
DMA-bound (~1456ns per 512-row pair at ~360GB/s).

Engine budgets per pair (cost model): DMA 1456, ACT 1416 (sq 612 +
sqrt 192 + copy1 612), DVE 1321 (reduce 594 + recip 69 + final 658),
Pool 1111 (prescale), PE ~660 (f16 transposes/matmuls), SP ~650.
"""

import numpy as np

B, N, DE, DN = 2, 512, 64, 128
NCORES = 8
IPC = N // NCORES          # 64 i-rows per core
NBLK = B * IPC             # 128 blocks of 512 rows per core
NPAIR = NBLK // 2
EPS = float(np.finfo(np.float32).eps)


def _build_nc(npair=NPAIR):
    from contextlib import ExitStack

    import concourse.bass as bass
    import concourse.mybir as mybir

    f32 = mybir.dt.float32
    f32r = mybir.dt.float32r
    f16 = mybir.dt.float16
    SQRT = mybir.ActivationFunctionType.Sqrt
    SQUARE = mybir.ActivationFunctionType.Square

    nc = bass.Bass()
    nblk = 2 * npair
    x_d = nc.declare_dram_parameter("x", [nblk, N, DE], f32, isOutput=False)
    colrm_d = nc.declare_dram_parameter("colrm", [128, 2 * 512], f16, isOutput=False)
    rp8_d = nc.declare_dram_parameter("rp8", [2, npair * 512], f16, isOutput=False)
    ind2c_d = nc.declare_dram_parameter("ind2c", [2, 128], f16, isOutput=False)
    wgblk_d = nc.declare_dram_parameter("wgblk", [128, 128], f16, isOutput=False)
    id128_d = nc.declare_dram_parameter("id128", [128, 128], f16, isOutput=False)
    out_d = nc.declare_dram_parameter("out", [nblk, N, DE], f16, isOutput=True)

    # buffer depths
    DXI = 6   # xin 2-pair slots (12 pairs of input runahead)
    DQ = 3    # sq
    DS = 3    # ss / rms / inv
    DXS = 4   # xs
    DXT = 4   # xT
    DO2 = 4   # outsb 2-pair slots
    DP1, DP2 = 4, 4   # psum banks (8 of 8)

    st = ExitStack()
    with st:
        sb = lambda shape, dt, name: st.enter_context(
            nc.sbuf_tensor(name, shape, dt)
        )
        colrm = sb([128, 1024], f16, "colrm_sb")
        rp8 = sb([2, npair * 512], f16, "rp8_sb")
        ind2c = sb([2, 128], f16, "ind2c_sb")
        wgblk = sb([128, 128], f16, "wgblk_sb")
        id128 = sb([128, 128], f16, "id128_sb")
        epsb = sb([128, 1], f32, "epsb")
        xin = [sb([128, 1024], f32, f"xin{i}") for i in range(DXI)]
        sq = [sb([128, 512], f32, f"sq{i}") for i in range(DQ)]
        ss = [sb([128, 8], f32, f"ss{i}") for i in range(DS)]
        rms = [sb([128, 8], f32, f"rms{i}") for i in range(DS)]
        inv = [sb([128, 8], f32, f"inv{i}") for i in range(DS)]
        xs = [sb([128, 512], f16, f"xs{i}") for i in range(DXS)]
        xT = [sb([128, 512], f16, f"xT{i}") for i in range(DXT)]
        outsb = [sb([128, 1024], f16, f"outsb{i}") for i in range(DO2)]
        ps1 = [
            st.enter_context(nc.psum_tensor(f"ps1{i}", [128, 512], f16))
            for i in range(DP1)
        ]
        ps2 = [
            st.enter_context(nc.psum_tensor(f"ps2{i}", [128, 512], f32))
            for i in range(DP2)
        ]

        sem = lambda name: st.enter_context(nc.semaphore(name))
        s_c = sem("s_c")
        s_c2 = sem("s_c2")
        s_in = [sem(f"s_in{i}") for i in range(DXI)]
        s_in0b = sem("s_in0b")
        s_out = [sem(f"s_out{i}") for i in range(DO2)]
        s_pl = sem("s_pl")
        s_dve = sem("s_dve")
        s_act = sem("s_act")
        s_pe = sem("s_pe")

        # Stage schedule (engine-iteration u for pair t) -- cross-engine
        # hops get a full iteration of slack EXCEPT two same-iteration
        # pairings (sqrt->recip, prescale->fwdT) that shorten the ladder:
        #   in-dma(t,t+1)@SP u=t (even t), sq(t)@ACT u=t,
        #   reduce(t)@DVE u=t+1, sqrt(t)@ACT u=t+2, recip(t)@DVE u=t+3,
        #   prescale(t)@Pool u=t+4, fwdT(t)@PE u=t+5, copy1(t)@ACT u=t+6,
        #   prime+mmj(t)@PE u=t+7, final(t)@DVE u=t+8,
        #   out-dma(t,t+1)@SP u=t+14 (even t).
        done = {}
        for t in range(npair):
            # slot 0's first fill is two 1-pair DMAs: pair0 -> s_in[0]+16,
            # pair1 -> s_in0b+16 (separate sems: concurrent DMAs must not
            # share one semaphore). Later fills: one 2-pair DMA +16.
            done[("in", t)] = 16 * ((t // 2) // DXI + 1)   # on s_in[(t//2)%DXI]
        done[("in", 1)] = 16                               # on s_in0b
        for k in range(npair // 2):
            done[("out", k)] = 16 * (k // DO2 + 1)         # on s_out[k%DO2]
        # s_act: per u: copy1(u-6), sqrt(u-2), sq(u)
        c = 0
        for u in range(npair + 6):
            if 6 <= u < npair + 6:
                c += 1; done[("copy1", u - 6)] = c
            if 2 <= u < npair + 2:
                c += 1; done[("sqrt", u - 2)] = c
            if u < npair:
                c += 1; done[("sq", u)] = c
        # s_dve: per u: final(u-8), recip(u-3), reduce(u-1)
        c = 0
        for u in range(npair + 8):
            if 8 <= u < npair + 8:
                c += 1; done[("final", u - 8)] = c
            if 3 <= u < npair + 3:
                c += 1; done[("recip", u - 3)] = c
            if 1 <= u < npair + 1:
                c += 1; done[("reduce", u - 1)] = c
        # s_pl: per u: prescale(u-4), sqb(u)
        c = 0
        for u in range(npair + 4):
            if 4 <= u < npair + 4:
                c += 1; done[("prescale", u - 4)] = c
            if u < npair:
                c += 1; done[("sqb", u)] = c
        # s_pe: per u: prime+mmj(u-7) +5, fwdT(u-5) +4
        c = 0
        for u in range(npair + 7):
            if 7 <= u < npair + 7:
                c += 5; done[("mmj", u - 7)] = c
            if 5 <= u < npair + 5:
                c += 4; done[("fwdT", u - 5)] = c

        CONST_TARGET = 4 * 16   # s_c: wgblk, id128, ind2c, rp8 (PE)
        COLRM_TARGET = 16       # s_c2: colrm (DVE final)

        def _pair2(ap):
            # 2 pairs (4 blocks) -> [(a p)=128, s=2, r=8, e=64]
            ap = ap.rearrange("(s a) (p r) e -> s a p r e", s=2, a=2, p=64, r=8)
            ap = ap.transpose([1, 2, 0, 3, 4])
            return ap.rearrange("a p s r e -> (a p) s r e")

        def _slot5(sb_slot):
            # [128, 1024] sbuf slot -> [128, s=2, r=8, e=64]
            return sb_slot[:].rearrange("q (s r e) -> q s r e", s=2, r=8)

        def in_src(t):
            return _pair2(x_d[2 * t : 2 * t + 4])

        def out_dst(t):
            return _pair2(out_d[2 * t : 2 * t + 4])

        def xin_view(t):
            return xin[(t // 2) % DXI][:, 512 * (t % 2) : 512 * (t % 2) + 512]

        def outsb_view(t):
            return outsb[(t // 2) % DO2][:, 512 * (t % 2) : 512 * (t % 2) + 512]

        with nc.Block() as block:

            @block.sync
            def _(sync):
                # first slot as two 1-pair DMAs so sq(0) starts sooner
                s5 = _slot5(xin[0])
                p1 = _pair2(x_d[0:4])
                sync.dma_start(
                    out=s5[:, 0:1], in_=p1[:, 0:1]
                ).then_inc(s_in[0], 16)
                sync.dma_start(
                    out=s5[:, 1:2], in_=p1[:, 1:2]
                ).then_inc(s_in0b, 16)
                sync.dma_start(
                    out=_slot5(xin[1]), in_=in_src(2)
                ).then_inc(s_in[1], 16)
                sync.dma_start(
                    out=_slot5(xin[2]), in_=in_src(4)
                ).then_inc(s_in[2], 16)
                for cdst, csrc in (
                    (ind2c, ind2c_d[:]),
                    (wgblk, wgblk_d[:]),
                    (id128, id128_d[:]),
                    (rp8, rp8_d[:]),
                ):
                    sync.dma_start(out=cdst[:], in_=csrc).then_inc(s_c, 16)
                sync.dma_start(out=colrm[:], in_=colrm_d[:]).then_inc(s_c2, 16)
                for u in range(npair + 14):
                    if 4 < u < npair and u % 2 == 0:
                        if u >= 2 * DXI:
                            sync.wait_ge(s_pl, done[("prescale", u - 2 * DXI + 1)])
                        sync.dma_start(
                            out=_slot5(xin[(u // 2) % DXI]), in_=in_src(u)
                        ).then_inc(s_in[(u // 2) % DXI], 16)
                    if u >= 14 and (u - 14) % 2 == 0 and (u - 14) < npair:
                        t = u - 14
                        sync.wait_ge(s_dve, done[("final", t + 1)])
                        sync.dma_start(
                            out=out_dst(t), in_=_slot5(outsb[(t // 2) % DO2])
                        ).then_inc(s_out[(t // 2) % DO2], 16)

            @block.vector
            def _(vector):
                nc.vector.memset(epsb[:], EPS)
                for u in range(npair + 8):
                    if 8 <= u < npair + 8:
                        t = u - 8
                        if u == 8:
                            vector.wait_ge(s_c2, COLRM_TARGET)
                        vector.wait_ge(s_pe, done[("mmj", t)])
                        if t // 2 >= DO2:
                            k = t // 2 - DO2
                            vector.wait_ge(s_out[k % DO2], done[("out", k)])
                        bslice = 512 * (t // (IPC // 2))
                        nc.vector.tensor_add(
                            outsb_view(t),
                            ps2[t % DP2][:],
                            colrm[:, bslice : bslice + 512],
                        ).then_inc(s_dve, 1)
                    if 3 <= u < npair + 3:
                        t = u - 3
                        vector.wait_ge(s_act, done[("sqrt", t)])
                        if t >= DS:
                            vector.wait_ge(s_pl, done[("prescale", t - DS)])
                        nc.vector.reciprocal(
                            inv[t % DS][:], rms[t % DS][:]
                        ).then_inc(s_dve, 1)
                    if 1 <= u < npair + 1:
                        t = u - 1
                        vector.wait_ge(s_act, done[("sq", t)])
                        vector.wait_ge(s_pl, done[("sqb", t)])
                        if t >= DS:
                            vector.wait_ge(s_act, done[("sqrt", t - DS)])
                        nc.vector.tensor_reduce(
                            ss[t % DS][:],
                            sq[t % DQ][:].rearrange("p (g e) -> p g e", e=DE),
                            axis=mybir.AxisListType.X,
                            op=mybir.AluOpType.add,
                        ).then_inc(s_dve, 1)

            @block.scalar
            def _(scalar):
                for u in range(npair + 6):
                    if 6 <= u < npair + 6:
                        t = u - 6
                        scalar.wait_ge(s_pe, done[("fwdT", t)])
                        if t >= DXT:
                            scalar.wait_ge(s_pe, done[("mmj", t - DXT)])
                        nc.scalar.copy(xT[t % DXT][:], ps1[t % DP1][:]).then_inc(
                            s_act, 1
                        )
                    if 2 <= u < npair + 2:
                        t = u - 2
                        scalar.wait_ge(s_dve, done[("reduce", t)])
                        if t >= DS:
                            scalar.wait_ge(s_dve, done[("recip", t - DS)])
                        nc.scalar.activation(
                            rms[t % DS][:], ss[t % DS][:], SQRT,
                            bias=epsb[:], scale=1.0 / DE,
                        ).then_inc(s_act, 1)
                    if u < npair:
                        if u == 1:
                            scalar.wait_ge(s_in0b, done[("in", 1)])
                        else:
                            scalar.wait_ge(s_in[(u // 2) % DXI], done[("in", u)])
                        if u >= DQ:
                            scalar.wait_ge(s_dve, done[("reduce", u - DQ)])
                        nc.scalar.activation(
                            sq[u % DQ][:, 0:436], xin_view(u)[:, 0:436], SQUARE,
                        ).then_inc(s_act, 1)

            @block.gpsimd
            def _(pool):
                for u in range(npair + 4):
                    if 4 <= u < npair + 4:
                        t = u - 4
                        pool.wait_ge(s_dve, done[("recip", t)])
                        if t >= DXS:
                            pool.wait_ge(s_pe, done[("fwdT", t - DXS)])
                        nc.gpsimd.tensor_mul(
                            xs[t % DXS][:].rearrange("p (g e) -> p g e", e=DE),
                            xin_view(t).rearrange("p (g e) -> p g e", e=DE),
                            inv[t % DS][:].unsqueeze(-1).broadcast_to([128, 8, DE]),
                        ).then_inc(s_pl, 1)
                    if u < npair:
                        if u == 1:
                            pool.wait_ge(s_in0b, done[("in", 1)])
                        else:
                            pool.wait_ge(
                                s_in[(u // 2) % DXI], done[("in", u)]
                            )
                        if u >= DQ:
                            pool.wait_ge(s_dve, done[("reduce", u - DQ)])
                        nc.gpsimd.tensor_mul(
                            sq[u % DQ][:, 436:512],
                            xin_view(u)[:, 436:512],
                            xin_view(u)[:, 436:512],
                        ).then_inc(s_pl, 1)

            @block.tensor
            def _(tensor):
                tensor.wait_ge(s_c, CONST_TARGET)
                for u in range(npair + 7):
                    if 7 <= u < npair + 7:
                        t = u - 7
                        tensor.wait_ge(s_act, done[("copy1", t)])
                        if t >= DP2:
                            tensor.wait_ge(s_dve, done[("final", t - DP2)])
                        nc.tensor.matmul(
                            ps2[t % DP2][:],
                            ind2c[:],
                            rp8[:, 512 * t : 512 * t + 512],
                            start=True, stop=False,
                        ).then_inc(s_pe, 1)
                        for q in range(4):
                            nc.tensor.matmul(
                                ps2[t % DP2][:, 128 * q : 128 * q + 128],
                                xT[t % DXT][:, 128 * q : 128 * q + 128],
                                wgblk[:],
                                start=False, stop=(q == 3),
                            ).then_inc(s_pe, 1)
                    if 5 <= u < npair + 5:
                        t = u - 5
                        tensor.wait_ge(s_pl, done[("prescale", t)])
                        if t >= DP1:
                            tensor.wait_ge(s_act, done[("copy1", t - DP1)])
                        for q in range(4):
                            nc.tensor.transpose(
                                ps1[t % DP1][:, 128 * q : 128 * q + 128],
                                xs[t % DXS][:, 128 * q : 128 * q + 128],
                                id128[:],
                            ).then_inc(s_pe, 1)

    return nc


_NC_CACHE = {}


def _get_nc():
    if "nc" not in _NC_CACHE:
        _NC_CACHE["nc"] = _build_nc()
    return _NC_CACHE["nc"]


def _make_in_maps(edges, nodes, g_node, g_edge, W, b):
    edges = np.ascontiguousarray(edges, dtype=np.float32)
    nodes = np.ascontiguousarray(nodes, dtype=np.float32)

    # ---- host: tiny node path (B*N*dn = 131K elems)
    ms = np.mean(np.square(nodes), axis=-1, keepdims=True)
    nodes_n = nodes / np.sqrt(ms + EPS) * g_node  # [B, N, 128]
    Wr, Wc, We = W[:DN], W[DN : 2 * DN], W[2 * DN :]
    row_proj = (nodes_n @ Wr).astype(np.float32)  # [B, N, 64]
    col_proj = (nodes_n @ Wc).astype(np.float32)  # [B, N, 64]
    Wg = (g_edge[:, None] * We).astype(np.float32)  # fold g_edge into We

    # colrm[c, 512b + (o,e)] = col_proj[b, 8*(c%64)+o, e] + bias
    cp = (col_proj + b).astype(np.float16).reshape(B, 64, 8 * DE)  # [2, 64, 512]
    colrm = np.concatenate([cp, cp], axis=1)  # [2, 128, 512]
    colrm = np.ascontiguousarray(colrm.transpose(1, 0, 2)).reshape(128, 2 * 512)

    # block-diagonal Wg (even r top-left, odd r bottom-right)
    wgblk = np.zeros((128, 128), dtype=np.float16)
    wgblk[:64, :64] = Wg.astype(np.float16)
    wgblk[64:, 64:] = Wg.astype(np.float16)
    id128 = np.eye(128, dtype=np.float16)
    # indicator: ind2c[k, m] = 1 if block-half m//64 == k
    ind2c = np.zeros((2, 128), dtype=np.float16)
    ind2c[0, :64] = 1.0
    ind2c[1, 64:] = 1.0

    in_maps = []
    for c in range(NCORES):
        xs = edges[:, c * IPC : (c + 1) * IPC]  # [B, 64, 512, 64]
        xs = np.ascontiguousarray(xs).reshape(NBLK, N, DE)
        # rp8[k, 512t + 64r + e] = row_proj[block 2t+k][e]  (tiled over r)
        rp = row_proj[:, c * IPC : (c + 1) * IPC].reshape(NBLK, DE)  # [128, 64]
        rp8 = np.empty((2, NPAIR, 8, DE), dtype=np.float16)
        rp8[0] = np.broadcast_to(rp[0::2, None, :], (NPAIR, 8, DE))
        rp8[1] = np.broadcast_to(rp[1::2, None, :], (NPAIR, 8, DE))
        rp8 = rp8.reshape(2, NPAIR * 512)
        in_maps.append(
            {
                "x": xs,
                "colrm": colrm,
                "rp8": rp8,
                "ind2c": ind2c,
                "wgblk": wgblk,
                "id128": id128,
            }
        )
    return in_maps


LAST_RESULT = None


def kernel(edges, nodes, g_node, g_edge, W, b):
    in_maps = _make_in_maps(edges, nodes, g_node, g_edge, W, b)

    from concourse.bass_utils import run_bass_kernel_spmd

    nc = _get_nc()
    res = run_bass_kernel_spmd(nc, in_maps, list(range(NCORES)))
    global LAST_RESULT
    LAST_RESULT = res

    out = np.empty((B, N, N, DE), dtype=np.float32)
    for c in range(NCORES):
        oc = res.results[c]["out"].astype(np.float32).reshape(B, IPC, N, DE)
        out[:, c * IPC : (c + 1) * IPC] = oc
    return out


if __name__ == "__main__":
    rng = np.random.default_rng(0)
    edges = rng.standard_normal((B, N, N, DE), dtype=np.float32)
    nodes = rng.standard_normal((B, N, DN), dtype=np.float32)
    g_node = np.ones(DN, np.float32)
    g_edge = np.ones(DE, np.float32)
    W = rng.standard_normal((2 * DN + DE, DE), dtype=np.float32) / 18.0
    b = (rng.standard_normal(DE) * 0.01).astype(np.float32)
    o = kernel(edges, nodes, g_node, g_edge, W, b)
    print(o.shape, o.dtype)
